# revision 9
# baseline (speedup 1.0000x reference)
"""NashLoss2D on 8 TRN2 NeuronCores — transposed f16 streaming design.

Inputs pred/targ are [10000, 5000] f32; targ has NaNs (missing obs).
Per station (column) j the loss needs four masked row-reductions:
    cnt_j = sum(~isnan(targ))
    s1_j  = sum(tz)        tz = targ | nan->0
    s2_j  = sum(tz^2)
    res_j = sum(dz^2)      dz = tz - (pred | targ-nan-lanes->0)
then scalar finalization (mean/sst/valid/per_col), O(NS), host f64.

Measured-cost design (HW probes; the repo cost model is optimistic for
accumulator ops):
- Host pre-transposes each core's 625-station slab to station-major
  [625, 10000] f16 (gate is 2e-2 rel err; f16 end-to-end error ~1e-6).
  DMA halves to 25 MB/core -> ~70 us at 360 GB/s.
- Stations on partitions (5 groups x 125), time on the free axis;
  reductions ride accum_out on instructions that build needed planes.
- NaN handling without copy_predicated (2762 ns/2.5k-tile) or Pool
  tensor_scalar (35 us!! software loop):
    * vm = (tg bitcast i16) <= 0x7C00  -> valid mask; fused accum = cnt
      (tensor_scalar+accum runs 1.05 cyc/elem on HW - the 4x mode only
      applies without an accumulator).
    * tgc = min(max(tg,-512),512): one PLAIN tensor_scalar (4x mode,
      ~0.3 cyc/elem); DVE max/min are IEEE maxNum/minNum so NaN -> -512.
    * tz = (tgc*1+0)*vm via affine_mul_reduce (custom DVE op): NaN lane
      = -512*0 = 0; its accum_out gives s1 for free.
- Engine balance per [125 x 5000] tile pair (10 tiles/core), measured
  ns/tile: DVE ts+acc 5340, ts-plain 1460, TT 4450, AMR ~5340;
  Pool TT 11100; ACT Square+acc 4600:
    DVE : vm, tgc, tz(AMR), dz on 6/10 tiles    ~150 us
    Pool: pz = pr*vm all tiles, dz on 4/10      ~150 us
    ACT : Square(tz)+acc, Square(dz)+acc        ~ 92 us
"""

import sys
from contextlib import ExitStack

import numpy as np

sys.path.insert(0, "/opt/trn_rl_repo")

import concourse.bass as bass  # noqa: E402
import concourse.tile as tile  # noqa: E402
from concourse import mybir  # noqa: E402
from concourse.bass_utils import run_bass_kernel_spmd  # noqa: E402

NT = 10000  # timesteps
NS = 5000  # stations
NCORES = 8
SC = NS // NCORES  # 625 stations per core
G = 5  # station groups per core (125 partitions each)
P = 125  # partitions (stations per group)
F = 5000  # time-chunk width (free axis)
C = NT // F  # 2 time chunks
NTILE = G * C  # 10 tiles per tensor per core

_NC_CACHE = {}


def _build_nc():
    nc = bass.Bass()
    f16 = mybir.dt.float16
    f32 = mybir.dt.float32
    i16 = mybir.dt.int16
    Act = mybir.ActivationFunctionType
    Op = mybir.AluOpType

    targ = nc.declare_dram_parameter("targ", [SC, NT], f16, isOutput=False)
    pred = nc.declare_dram_parameter("pred", [SC, NT], f16, isOutput=False)
    out = nc.declare_dram_parameter("out", [P, 40], f32, isOutput=True)

    with ExitStack() as ctx:
        tc = ctx.enter_context(tile.TileContext(nc))
        singles = ctx.enter_context(tc.tile_pool(name="singles", bufs=1))
        work = ctx.enter_context(tc.tile_pool(name="work", bufs=2))

        # accum slots split by writing engine (no cross-engine tile deps).
        # dve_acc cols: cnt at slot, s1 at 10+slot; act_acc: s2, res.
        dve_acc = singles.tile([P, 20], f32)
        act_acc = singles.tile([P, 20], f32)

        for g in range(G):
            for c in range(C):
                slot = g * C + c
                tg = work.tile([P, F], f16, tag="tg")
                pr = work.tile([P, F], f16, tag="pr")
                vm = work.tile([P, F], f16, tag="vm")
                tgc = work.tile([P, F], f16, tag="tgc")
                tz = work.tile([P, F], f16, tag="tz")
                pz = work.tile([P, F], f16, tag="pz")
                dz = work.tile([P, F], f16, tag="dz")
                sq = work.tile([P, F], f16, tag="sq")

                r0, t0 = g * P, c * F
                nc.sync.dma_start(out=tg, in_=targ[r0 : r0 + P, t0 : t0 + F])
                nc.sync.dma_start(out=pr, in_=pred[r0 : r0 + P, t0 : t0 + F])

                # vm = 1.0 at valid lanes (f16 NaNs are > 0x7C00 as int16);
                # fused accum -> cnt
                nc.vector.tensor_scalar(
                    out=vm, in0=tg[:].bitcast(i16), scalar1=31744.0, op0=Op.is_le,
                    scalar2=None, op1=Op.add, accum_out=dve_acc[:, slot : slot + 1],
                )
                # tgc: NaN -> -512 (IEEE maxNum), valid values untouched;
                # plain tensor_scalar runs in the DVE 4x perf mode
                nc.vector.tensor_scalar(
                    out=tgc, in0=tg, scalar1=-512.0, op0=Op.max,
                    scalar2=512.0, op1=Op.min,
                )
                # pz = pred masked to targ's valid lanes (Pool, overlaps DVE)
                nc.gpsimd.tensor_tensor(pz, pr, vm, Op.mult)
                # tz = tgc * vm (NaN lane: -512*0 = 0); accum -> s1
                nc.vector.affine_mul_reduce(
                    out=tz, accum_out=dve_acc[:, 10 + slot : 11 + slot],
                    in0=tgc, in1=vm, scale=1.0, bias=0.0,
                )
                # dz = tz - pz; 4 of 10 tiles on Pool to balance engines
                if slot % 5 in (1, 3):
                    nc.gpsimd.tensor_tensor(dz, tz, pz, Op.subtract)
                else:
                    nc.vector.tensor_tensor(dz, tz, pz, Op.subtract)
                nc.scalar.activation(
                    sq, tz, Act.Square, accum_out=act_acc[:, slot : slot + 1]
                )
                nc.scalar.activation(
                    sq, dz, Act.Square, accum_out=act_acc[:, 10 + slot : 11 + slot]
                )

        nc.sync.dma_start(out=out[:, 0:20], in_=dve_acc)
        nc.sync.dma_start(out=out[:, 20:40], in_=act_acc)

    import bass_rust as _bass_rust
    from concourse.library_overlay import lower_extended_insts

    # raw Bass skips Bacc's codegen_inst_isa_subclasses pass; without it the
    # custom-DVE InstISA instructions have empty .instr -> "ISA wrong length"
    lower_extended_insts(nc)
    _bass_rust.generate_event_semaphores(nc)
    return nc


def get_nc():
    if "nc" not in _NC_CACHE:
        _NC_CACHE["nc"] = _build_nc()
    return _NC_CACHE["nc"]


def make_in_maps(pred: np.ndarray, targ: np.ndarray) -> list:
    in_maps = []
    for c in range(NCORES):
        sl = slice(c * SC, (c + 1) * SC)
        in_maps.append(
            {
                "pred": np.ascontiguousarray(pred[:, sl].T).astype(np.float16),
                "targ": np.ascontiguousarray(targ[:, sl].T).astype(np.float16),
            }
        )
    return in_maps


def _unpack(raw: np.ndarray) -> np.ndarray:
    """[125, 40] device accum slots -> [4, SC] stats (cnt, s1, s2, res).

    Column layout: [cnt(0:10) | s1(10:20) | s2(20:30) | res(30:40)],
    slot = g*C + c. Station index = g*125 + p.
    """

    def blk(j):
        b = raw[:, j * 10 : (j + 1) * 10].astype(np.float64)
        return b.reshape(P, G, C).sum(axis=2).T.reshape(SC)  # s = g*125 + p

    return np.stack([blk(0), blk(1), blk(2), blk(3)])


def _finalize(stats: np.ndarray) -> np.ndarray:
    """stats: [4, NS] f64 (cnt, s1, s2, res) -> scalar f32 loss (host, f64)."""
    cnt, s1, s2, res = stats
    cntf = np.maximum(cnt, 1.0)
    mean = s1 / cntf
    sst = s2 - s1 * mean
    valid = (cnt > 10) & (sst != 0.0)
    sst_safe = np.where(valid, np.maximum(sst, 0.0), 1.0)
    per_col = np.where(valid, res / (np.sqrt(sst_safe) + 0.1) ** 2, 0.0)
    n = valid.sum()
    return np.array(per_col.sum() / n, dtype=np.float32)


def finalize_results(results: list) -> np.ndarray:
    stats = np.concatenate([_unpack(r["out"]) for r in results], axis=1)  # [4, NS]
    return _finalize(stats)


def kernel(pred: np.ndarray, targ: np.ndarray) -> np.ndarray:
    nc = get_nc()
    in_maps = make_in_maps(pred, targ)
    try:
        results = run_bass_kernel_spmd(nc, in_maps, list(range(NCORES))).results
    except Exception:
        # tile scheduling is not perfectly deterministic across processes; a
        # rebuild gives a fresh schedule if a rare bad one failed to compile
        _NC_CACHE.clear()
        nc = get_nc()
        results = run_bass_kernel_spmd(nc, in_maps, list(range(NCORES))).results
    return finalize_results(results)


# revision 11
# speedup vs baseline: 1.2194x; 1.2194x over previous
"""NashLoss2D on 8 TRN2 NeuronCores — transposed f16 streaming design.

Inputs pred/targ are [10000, 5000] f32; targ has NaNs (missing obs).
Per station (column) j the loss needs four masked row-reductions:
    cnt_j = sum(~isnan(targ))
    s1_j  = sum(tz)        tz = targ | nan->0
    s2_j  = sum(tz^2)
    res_j = sum(dz^2)      dz = tz - (pred | targ-nan-lanes->0)
then scalar finalization (mean/sst/valid/per_col), O(NS), host f64.

Design driven by HW-measured instruction rates (ns per element-line;
the repo cost model is wrong for several of these):
    DVE 1-stage plain tensor_scalar   0.31   (4x perf mode)
    DVE 2-stage plain tensor_scalar   2.15   (4x mode lost!)
    DVE tensor_tensor (f16 2x_1p)     0.66
    DVE tensor_scalar + accum         1.28   (accum kills fast modes)
    ACT activation + accum            1.07   (2 engine-cap passes... 3 used)
    Pool tensor_tensor                2.9    (only worth one pass)
    Pool tensor_scalar               13.7    (software loop - never)
Key structure:
- Host pre-transposes each core's 625-station slab to station-major
  [625, 10000] f16 (tolerance 2e-2; measured end-to-end error ~1e-7).
  DMA: 25 MB/core -> ~70 us.
- Stations on partitions (5 groups x 125), time on free axis.
- NaN masking uses IEEE maxNum/minNum (probed: max/min(NaN,0)=0):
  tz = max(tg,0) + min(tg,0) via two 1-stage plain ts + one TT.
- vm = (tg bitcast i16) <= 0x7C00 with fused accum -> cnt.
- s1 rides a third ACT pass (Copy(tz)+accum, tableless).
Engine totals (us/core): DVE ~161, ACT ~161, Pool ~145, DMA ~70.
"""

import sys
from contextlib import ExitStack

import numpy as np

sys.path.insert(0, "/opt/trn_rl_repo")

import concourse.bass as bass  # noqa: E402
import concourse.tile as tile  # noqa: E402
from concourse import mybir  # noqa: E402
from concourse.bass_utils import run_bass_kernel_spmd  # noqa: E402

NT = 10000  # timesteps
NS = 5000  # stations
NCORES = 8
SC = NS // NCORES  # 625 stations per core
G = 5  # station groups per core (125 partitions each)
P = 125  # partitions (stations per group)
F = 5000  # time-chunk width (free axis)
C = NT // F  # 2 time chunks
NTILE = G * C  # 10 tiles per tensor per core

_NC_CACHE = {}


def _build_nc():
    nc = bass.Bass()
    f16 = mybir.dt.float16
    f32 = mybir.dt.float32
    i16 = mybir.dt.int16
    Act = mybir.ActivationFunctionType
    Op = mybir.AluOpType

    targ = nc.declare_dram_parameter("targ", [SC, NT], f16, isOutput=False)
    pred = nc.declare_dram_parameter("pred", [SC, NT], f16, isOutput=False)
    out = nc.declare_dram_parameter("out", [P, 40], f32, isOutput=True)

    with ExitStack() as ctx:
        tc = ctx.enter_context(tile.TileContext(nc))
        singles = ctx.enter_context(tc.tile_pool(name="singles", bufs=1))
        work = ctx.enter_context(tc.tile_pool(name="work", bufs=2))

        # accum slots split by writing engine (no cross-engine tile deps).
        # dve_acc col: cnt at slot; act_acc: s1 at slot, s2 at 10+, res at 20+
        dve_acc = singles.tile([P, 10], f32)
        act_acc = singles.tile([P, 30], f32)

        for g in range(G):
            for c in range(C):
                slot = g * C + c
                tg = work.tile([P, F], f16, tag="tg")
                pr = work.tile([P, F], f16, tag="pr")
                vm = work.tile([P, F], f16, tag="vm")
                m1 = work.tile([P, F], f16, tag="m1")
                m2 = work.tile([P, F], f16, tag="m2")
                tz = work.tile([P, F], f16, tag="tz")
                pz = work.tile([P, F], f16, tag="pz")
                dz = work.tile([P, F], f16, tag="dz")
                sq = work.tile([P, F], f16, tag="sq")

                r0, t0 = g * P, c * F
                nc.sync.dma_start(out=tg, in_=targ[r0 : r0 + P, t0 : t0 + F])
                nc.sync.dma_start(out=pr, in_=pred[r0 : r0 + P, t0 : t0 + F])

                # vm = 1.0 at valid lanes (f16 NaNs are > 0x7C00 as int16);
                # fused accum -> cnt. Goes first so Pool's pz can start early.
                nc.vector.tensor_scalar(
                    out=vm, in0=tg[:].bitcast(i16), scalar1=31744.0, op0=Op.is_le,
                    scalar2=None, op1=Op.add, accum_out=dve_acc[:, slot : slot + 1],
                )
                # halves: NaN -> 0 per IEEE maxNum/minNum; 1-stage plain ts
                # runs in the DVE 4x perf mode
                nc.vector.tensor_scalar(
                    out=m1, in0=tg, scalar1=0.0, op0=Op.max,
                    scalar2=0.0, op1=Op.bypass,
                )
                nc.vector.tensor_scalar(
                    out=m2, in0=tg, scalar1=0.0, op0=Op.min,
                    scalar2=0.0, op1=Op.bypass,
                )
                # pz = pred masked to targ's valid lanes (Pool's one pass)
                nc.gpsimd.tensor_tensor(pz, pr, vm, Op.mult)
                nc.vector.tensor_tensor(tz, m1, m2, Op.add)
                nc.vector.tensor_tensor(dz, tz, pz, Op.subtract)
                nc.scalar.activation(
                    sq, tz, Act.Square, accum_out=act_acc[:, 10 + slot : 11 + slot]
                )
                # s1 = sum(tz) rides ACT's third pass; Copy is tableless
                nc.scalar.activation(
                    sq, tz, Act.Copy, accum_out=act_acc[:, slot : slot + 1]
                )
                nc.scalar.activation(
                    sq, dz, Act.Square, accum_out=act_acc[:, 20 + slot : 21 + slot]
                )

        nc.sync.dma_start(out=out[:, 0:10], in_=dve_acc)
        nc.sync.dma_start(out=out[:, 10:40], in_=act_acc)

    import bass_rust as _bass_rust
    from concourse.library_overlay import lower_extended_insts

    # raw Bass skips Bacc's codegen_inst_isa_subclasses pass; without it any
    # custom/ISA instruction has empty .instr -> "ISA wrong length"
    lower_extended_insts(nc)
    _bass_rust.generate_event_semaphores(nc)
    return nc


def get_nc():
    if "nc" not in _NC_CACHE:
        _NC_CACHE["nc"] = _build_nc()
    return _NC_CACHE["nc"]


def make_in_maps(pred: np.ndarray, targ: np.ndarray) -> list:
    in_maps = []
    for c in range(NCORES):
        sl = slice(c * SC, (c + 1) * SC)
        in_maps.append(
            {
                "pred": np.ascontiguousarray(pred[:, sl].T).astype(np.float16),
                "targ": np.ascontiguousarray(targ[:, sl].T).astype(np.float16),
            }
        )
    return in_maps


def _unpack(raw: np.ndarray) -> np.ndarray:
    """[125, 40] device accum slots -> [4, SC] stats (cnt, s1, s2, res).

    Column layout: [cnt(0:10) | s1(10:20) | s2(20:30) | res(30:40)],
    slot = g*C + c. Station index = g*125 + p.
    """

    def blk(j):
        b = raw[:, j * 10 : (j + 1) * 10].astype(np.float64)
        return b.reshape(P, G, C).sum(axis=2).T.reshape(SC)  # s = g*125 + p

    return np.stack([blk(0), blk(1), blk(2), blk(3)])


def _finalize(stats: np.ndarray) -> np.ndarray:
    """stats: [4, NS] f64 (cnt, s1, s2, res) -> scalar f32 loss (host, f64)."""
    cnt, s1, s2, res = stats
    cntf = np.maximum(cnt, 1.0)
    mean = s1 / cntf
    sst = s2 - s1 * mean
    valid = (cnt > 10) & (sst != 0.0)
    sst_safe = np.where(valid, np.maximum(sst, 0.0), 1.0)
    per_col = np.where(valid, res / (np.sqrt(sst_safe) + 0.1) ** 2, 0.0)
    n = valid.sum()
    return np.array(per_col.sum() / n, dtype=np.float32)


def finalize_results(results: list) -> np.ndarray:
    stats = np.concatenate([_unpack(r["out"]) for r in results], axis=1)  # [4, NS]
    return _finalize(stats)


def kernel(pred: np.ndarray, targ: np.ndarray) -> np.ndarray:
    nc = get_nc()
    in_maps = make_in_maps(pred, targ)
    try:
        results = run_bass_kernel_spmd(nc, in_maps, list(range(NCORES))).results
    except Exception:
        # tile scheduling is not perfectly deterministic across processes; a
        # rebuild gives a fresh schedule if a rare bad one failed to compile
        _NC_CACHE.clear()
        nc = get_nc()
        results = run_bass_kernel_spmd(nc, in_maps, list(range(NCORES))).results
    return finalize_results(results)
